# revision 9
# baseline (speedup 1.0000x reference)
"""CrossModalAttention TRN2 kernel.

Computation (per batch b):
  Q_m = x_m @ W_m ; K_m = x_m @ W_m^T   (m in {rna, cnv, clinical})
  out  = mean_i( sum_{j!=i} softmax(Q_i K_j^T / 8) @ x_j )

Strategy:
  - Pure data parallel: batch dim 16 sharded 2-per-core across 8 NeuronCores.
  - Tiny Q/K projections (1.6 GFLOP total) are precomputed on host; the device
    kernel runs the O(N^2) attention (103 GFLOP + 403M exps), which dominates.
  - Scores are computed transposed (ST[m, n] tiles) so the PV matmul contracts
    over m on the partition dim.  K=64 contraction -> two concurrent
    tile_position matmuls (rows 0-63 / 64-127) with Q,K duplicated across
    both partition halves.
  - exp on ScalarE reads score tiles straight from PSUM ([128, 1024] per
    instruction), with the 1/sqrt(64) scale folded into the activation.
    No max-subtraction: scores are ~N(0,1), max over 4e8 samples < 7, far
    from fp32 exp overflow.
  - Softmax denominator rides along as a 65th "feature" column of x_j set to
    3.0, so the PV matmul accumulates [65, 512] = [out^T ; 3*Z].  The 3.0
    folds the final mean-over-3-modalities into the existing normalization.
  - out^T chunks are PE-transposed back to [n, d] layout, normalized with a
    per-partition reciprocal scalar on VectorE, and accumulated over the 6
    (i, j) pairs in SBUF.
"""

import os

import numpy as np

import concourse.bass as bass
import concourse.bacc as bacc
import concourse.tile as tile
from concourse import mybir
from concourse.bass_utils import run_bass_kernel_spmd

B, N, D = 16, 2048, 64
NCORES = 8
BPC = B // NCORES  # batches per core
NT = N // 128  # 16 row-tiles of 128
CH = 512  # n-chunk (PSUM bank)
NCH = N // CH  # 4
PAIRS = [(i, j) for i in range(3) for j in range(3) if i != j]
SCALE = 1.0 / 8.0  # 1/sqrt(D)
F32 = mybir.dt.float32
F16 = mybir.dt.float16  # matmul operand dtype: 1 cyc/row, 10-bit mantissa

_cache = {}
last_results = None  # BassKernelResults of the most recent run (for test.py)


def _build():
    nc = bacc.Bacc()
    qt_d = [
        nc.declare_dram_parameter(f"qt{m}", [BPC, 128, N], F16, isOutput=False)
        for m in range(3)
    ]
    kt_d = [
        nc.declare_dram_parameter(f"kt{m}", [BPC, 128, N], F16, isOutput=False)
        for m in range(3)
    ]
    xo_d = [
        nc.declare_dram_parameter(f"xo{m}", [BPC, 128, NT, D + 1], F16, isOutput=False)
        for m in range(3)
    ]
    id_d = nc.declare_dram_parameter("ident", [128, 128], F32, isOutput=False)
    out_d = nc.declare_dram_parameter("out", [BPC, N, D], F32, isOutput=True)

    from contextlib import ExitStack

    with tile.TileContext(nc) as tc, ExitStack() as ctx:
        singles = ctx.enter_context(tc.tile_pool(name="singles", bufs=1))
        big = ctx.enter_context(tc.tile_pool(name="big", bufs=2))
        work = ctx.enter_context(tc.tile_pool(name="work", bufs=3))
        psum = ctx.enter_context(tc.tile_pool(name="psum", bufs=2, space="PSUM"))

        id_sb = singles.tile([128, 128], F32)
        nc.sync.dma_start(out=id_sb, in_=id_d[:, :])
        # Warm up the ACT engine: absorb the exp-table load and the const
        # bias-AP DMA wait into one early instruction, so the first real exp
        # (which also waits on PE) stays within the single ACT wait slot.
        warm = singles.tile([128, 1], F32)
        bias0 = nc.const_aps.scalar_like(0.0, warm[:, 0:1])
        nc.scalar.activation(warm, bias0, mybir.ActivationFunctionType.Exp)

        for b in range(BPC):
            qt_sb, kt_sb, xo_sb = [None] * 3, [None] * 3, [None] * 3
            for m in range(3):
                qt_sb[m] = big.tile([128, N], F16, tag=f"qt{m}", name=f"qt{m}_{b}")
                kt_sb[m] = big.tile([128, N], F16, tag=f"kt{m}", name=f"kt{m}_{b}")
                xo_sb[m] = big.tile(
                    [128, NT, D + 1], F16, tag=f"xo{m}", name=f"xo{m}_{b}"
                )
            # Issue the first pair's ((0,1)) inputs first so compute can start
            # before the remaining loads land.
            for m, t_sb, t_d in (
                (0, qt_sb, qt_d), (1, kt_sb, kt_d), (1, xo_sb, xo_d),
                (1, qt_sb, qt_d), (2, kt_sb, kt_d), (2, xo_sb, xo_d),
                (2, qt_sb, qt_d), (0, kt_sb, kt_d), (0, xo_sb, xo_d),
            ):
                nc.sync.dma_start(out=t_sb[m], in_=t_d[m][b])
            acc = big.tile([128, NT, D], F32, tag="acc", name=f"acc_{b}")
            nc.vector.memset(acc, 0.0)
            GROUPS = [3, 3, 3, 3, 3, 1]
            for (i, j) in PAIRS:
                for c in range(NCH):
                    out_ps = psum.tile([D + 1, CH], F32, tag="out", bufs=1, name=f"o_{b}_{i}{j}_{c}")
                    st_g, pt_g = [None] * len(GROUPS), [None] * len(GROUPS)

                    def st_slot(t):
                        g, p = t // 3, t % 3
                        if st_g[g] is None:
                            st_g[g] = psum.tile(
                                [128, GROUPS[g] * CH], F32, tag="st",
                                name=f"st_{b}_{i}{j}_{c}_{g}",
                            )
                        return st_g[g][:, p * CH : (p + 1) * CH]

                    for mp in range(NT // 2):
                        mA, mB = 2 * mp, 2 * mp + 1
                        # Two concurrent K=64 matmuls via partition halves:
                        # rows 0-63 compute m-tile mA, rows 64-127 m-tile mB.
                        nc.tensor.matmul(
                            st_slot(mA),
                            lhsT=(kt_sb[j][0:64, mA * 128 : (mA + 1) * 128]),
                            rhs=(qt_sb[i][0:64, c * CH : (c + 1) * CH]),
                            start=True,
                            stop=True,
                        )
                        nc.tensor.matmul(
                            st_slot(mB),
                            lhsT=(kt_sb[j][64:128, mB * 128 : (mB + 1) * 128]),
                            rhs=(qt_sb[i][64:128, c * CH : (c + 1) * CH]),
                            start=True,
                            stop=True,
                        )
                        for g in range(mA // 3, mB // 3 + 1):
                            g_hi = g * 3 + GROUPS[g] - 1  # last m-tile of group
                            if g_hi not in (mA, mB):
                                continue  # group not fully written yet
                            pt_g[g] = work.tile(
                                [128, GROUPS[g] * CH], F16, tag="pt",
                                name=f"pt_{b}_{i}{j}_{c}_{g}",
                            )
                            nc.scalar.activation(
                                pt_g[g], st_g[g],
                                mybir.ActivationFunctionType.Exp, scale=SCALE,
                            )
                            for p in range(GROUPS[g]):
                                t = g * 3 + p
                                nc.tensor.matmul(
                                    out_ps,
                                    lhsT=(xo_sb[j][:, t, :]),
                                    rhs=(pt_g[g][:, p * CH : (p + 1) * CH]),
                                    start=(t == 0),
                                    stop=(t == NT - 1),
                                    skip_group_check=True,
                                )
                    # out_ps rows 0-63 = unnormalized out^T, row 64 = 3*Z.
                    osb = work.tile([D + 1, CH], F32, tag="osb", name=f"osb_{b}_{i}{j}_{c}")
                    nc.vector.tensor_copy(out=osb, in_=out_ps)
                    otp = psum.tile([128, 4, D + 1], F32, tag="otp", bufs=1, name=f"otp_{b}_{i}{j}_{c}")
                    for t in range(4):
                        nc.tensor.transpose(
                            otp[:, t, :],
                            osb[:, t * 128 : (t + 1) * 128],
                            id_sb[0 : D + 1, 0 : D + 1],
                        )
                    rz = work.tile([128, 4], F32, tag="rz", name=f"rz_{b}_{i}{j}_{c}")
                    nc.vector.reciprocal(rz, otp[:, :, D])
                    res = work.tile([128, 4, D], F32, tag="res", name=f"res_{b}_{i}{j}_{c}")
                    for t in range(4):
                        nc.vector.tensor_scalar_mul(
                            res[:, t, :], otp[:, t, 0:D], rz[:, t : t + 1]
                        )
                    nc.vector.tensor_tensor(
                        out=acc[:, c * 4 : (c + 1) * 4, :],
                        in0=acc[:, c * 4 : (c + 1) * 4, :],
                        in1=res,
                        op=mybir.AluOpType.add,
                    )
            nc.sync.dma_start(
                out=out_d[b].rearrange("(t p) d -> p t d", p=128), in_=acc
            )
    nc.finalize()  # Bacc: split multi-waits, alloc regs, etc.
    return nc


def _prep(xs, Ws):
    """Host-side input prep: Q/K projections + layout shuffles."""
    qts, kts, xos = [], [], []
    for m in range(3):
        x = np.ascontiguousarray(xs[m], dtype=np.float32)  # [B, N, D]
        W = np.asarray(Ws[m], dtype=np.float32)
        Q = x @ W  # [B, N, D]
        K = x @ W.T
        QT = np.ascontiguousarray(Q.transpose(0, 2, 1))  # [B, D, N]
        KT = np.ascontiguousarray(K.transpose(0, 2, 1))
        qts.append(np.concatenate([QT, QT], axis=1).astype(np.float16))  # [B, 128, N]
        kts.append(np.concatenate([KT, KT], axis=1).astype(np.float16))
        xo = np.full((B, 128, NT, D + 1), 3.0, dtype=np.float16)
        # xo[b, p, t, :64] = x[b, t*128 + p, :]; col 64 stays 3.0 (folds the
        # mean over 3 modalities into the softmax normalization).
        xo[..., :D] = x.reshape(B, NT, 128, D).transpose(0, 2, 1, 3).astype(np.float16)
        xos.append(xo)
    return qts, kts, xos


def kernel(x_rna, x_cnv, x_clinical, W_rna, W_cnv, W_clinical):
    global last_results
    xs = [x_rna, x_cnv, x_clinical]
    Ws = [W_rna, W_cnv, W_clinical]
    qts, kts, xos = _prep(xs, Ws)
    ident = np.eye(128, dtype=np.float32)

    if "nc" not in _cache:
        _cache["nc"] = _build()
    nc = _cache["nc"]

    in_maps = []
    for c in range(NCORES):
        sl = slice(c * BPC, (c + 1) * BPC)
        m = {"ident": ident}
        for mod in range(3):
            m[f"qt{mod}"] = np.ascontiguousarray(qts[mod][sl])
            m[f"kt{mod}"] = np.ascontiguousarray(kts[mod][sl])
            m[f"xo{mod}"] = np.ascontiguousarray(xos[mod][sl])
        in_maps.append(m)

    last_results = run_bass_kernel_spmd(
        nc,
        in_maps,
        list(range(NCORES)),
        trace=bool(os.environ.get("BASS_TRACE")),
    )
    out = np.concatenate([r["out"] for r in last_results.results], axis=0)
    return out


# revision 10
# speedup vs baseline: 1.0041x; 1.0041x over previous
"""CrossModalAttention TRN2 kernel.

Computation (per batch b):
  Q_m = x_m @ W_m ; K_m = x_m @ W_m^T   (m in {rna, cnv, clinical})
  out  = mean_i( sum_{j!=i} softmax(Q_i K_j^T / 8) @ x_j )

Strategy:
  - Pure data parallel: batch dim 16 sharded 2-per-core across 8 NeuronCores.
  - Tiny Q/K projections (1.6 GFLOP total) are precomputed on host; the device
    kernel runs the O(N^2) attention (103 GFLOP + 403M exps), which dominates.
  - Scores are computed transposed (ST[m, n] tiles) so the PV matmul contracts
    over m on the partition dim.  K=64 contraction -> two concurrent
    tile_position matmuls (rows 0-63 / 64-127) with Q,K duplicated across
    both partition halves.
  - exp on ScalarE reads score tiles straight from PSUM ([128, 1024] per
    instruction), with the 1/sqrt(64) scale folded into the activation.
    No max-subtraction: scores are ~N(0,1), max over 4e8 samples < 7, far
    from fp32 exp overflow.
  - Softmax denominator rides along as a 65th "feature" column of x_j set to
    3.0, so the PV matmul accumulates [65, 512] = [out^T ; 3*Z].  The 3.0
    folds the final mean-over-3-modalities into the existing normalization.
  - out^T chunks are PE-transposed back to [n, d] layout, normalized with a
    per-partition reciprocal scalar on VectorE, and accumulated over the 6
    (i, j) pairs in SBUF.
"""

import os

import numpy as np

import concourse.bass as bass
import concourse.bacc as bacc
import concourse.tile as tile
from concourse import mybir
from concourse.bass_utils import run_bass_kernel_spmd

B, N, D = 16, 2048, 64
NCORES = 8
BPC = B // NCORES  # batches per core
NT = N // 128  # 16 row-tiles of 128
CH = 512  # n-chunk (PSUM bank)
NCH = N // CH  # 4
PAIRS = [(i, j) for i in range(3) for j in range(3) if i != j]
SCALE = 1.0 / 8.0  # 1/sqrt(D)
F32 = mybir.dt.float32
F16 = mybir.dt.float16  # matmul operand dtype: 1 cyc/row, 10-bit mantissa

_cache = {}
last_results = None  # BassKernelResults of the most recent run (for test.py)


def _build():
    nc = bacc.Bacc()
    qt_d = [
        nc.declare_dram_parameter(f"qt{m}", [BPC, 128, N], F16, isOutput=False)
        for m in range(3)
    ]
    kt_d = [
        nc.declare_dram_parameter(f"kt{m}", [BPC, 128, N], F16, isOutput=False)
        for m in range(3)
    ]
    xo_d = [
        nc.declare_dram_parameter(f"xo{m}", [BPC, 128, NT, D + 1], F16, isOutput=False)
        for m in range(3)
    ]
    id_d = nc.declare_dram_parameter("ident", [128, 128], F32, isOutput=False)
    out_d = nc.declare_dram_parameter("out", [BPC, N, D], F32, isOutput=True)

    from contextlib import ExitStack

    with tile.TileContext(nc) as tc, ExitStack() as ctx:
        singles = ctx.enter_context(tc.tile_pool(name="singles", bufs=1))
        big = ctx.enter_context(tc.tile_pool(name="big", bufs=2))
        work = ctx.enter_context(tc.tile_pool(name="work", bufs=3))
        psum = ctx.enter_context(tc.tile_pool(name="psum", bufs=2, space="PSUM"))

        id_sb = singles.tile([128, 128], F32)
        nc.sync.dma_start(out=id_sb, in_=id_d[:, :])
        # Warm up the ACT engine: absorb the exp-table load and the const
        # bias-AP DMA wait into one early instruction, so the first real exp
        # (which also waits on PE) stays within the single ACT wait slot.
        warm = singles.tile([128, 1], F32)
        bias0 = nc.const_aps.scalar_like(0.0, warm[:, 0:1])
        nc.scalar.activation(warm, bias0, mybir.ActivationFunctionType.Exp)

        for b in range(BPC):
            qt_sb, kt_sb, xo_sb = [None] * 3, [None] * 3, [None] * 3
            for m in range(3):
                qt_sb[m] = big.tile([128, N], F16, tag=f"qt{m}", name=f"qt{m}_{b}")
                kt_sb[m] = big.tile([128, N], F16, tag=f"kt{m}", name=f"kt{m}_{b}")
                xo_sb[m] = big.tile(
                    [128, NT, D + 1], F16, tag=f"xo{m}", name=f"xo{m}_{b}"
                )
            # Issue the first pair's ((0,1)) inputs first so compute can start
            # before the remaining loads land.
            for m, t_sb, t_d in (
                (0, qt_sb, qt_d), (1, kt_sb, kt_d), (1, xo_sb, xo_d),
                (1, qt_sb, qt_d), (2, kt_sb, kt_d), (2, xo_sb, xo_d),
                (2, qt_sb, qt_d), (0, kt_sb, kt_d), (0, xo_sb, xo_d),
            ):
                nc.sync.dma_start(out=t_sb[m], in_=t_d[m][b])
            acc = big.tile([128, NT, D], F32, tag="acc", name=f"acc_{b}")
            nc.vector.memset(acc, 0.0)
            GROUPS = [3, 3, 3, 3, 3, 1]
            pending = []  # deferred per-chunk normalize work (PE transposes
            # + DVE normalize), flushed after the NEXT chunk's first score
            # group is emitted so PE never starves the ACT exp stream.

            def flush_pending():
                while pending:
                    pending.pop(0)()

            for (i, j) in PAIRS:
                for c in range(NCH):
                    out_ps = psum.tile([D + 1, CH], F32, tag="out", bufs=1, name=f"o_{b}_{i}{j}_{c}")
                    st_g, pt_g = [None] * len(GROUPS), [None] * len(GROUPS)

                    def st_slot(t):
                        g, p = t // 3, t % 3
                        if st_g[g] is None:
                            st_g[g] = psum.tile(
                                [128, GROUPS[g] * CH], F32, tag="st",
                                name=f"st_{b}_{i}{j}_{c}_{g}",
                            )
                        return st_g[g][:, p * CH : (p + 1) * CH]

                    for mp in range(NT // 2):
                        mA, mB = 2 * mp, 2 * mp + 1
                        # Two concurrent K=64 matmuls via partition halves:
                        # rows 0-63 compute m-tile mA, rows 64-127 m-tile mB.
                        nc.tensor.matmul(
                            st_slot(mA),
                            lhsT=(kt_sb[j][0:64, mA * 128 : (mA + 1) * 128]),
                            rhs=(qt_sb[i][0:64, c * CH : (c + 1) * CH]),
                            start=True,
                            stop=True,
                        )
                        nc.tensor.matmul(
                            st_slot(mB),
                            lhsT=(kt_sb[j][64:128, mB * 128 : (mB + 1) * 128]),
                            rhs=(qt_sb[i][64:128, c * CH : (c + 1) * CH]),
                            start=True,
                            stop=True,
                        )
                        for g in range(mA // 3, mB // 3 + 1):
                            g_hi = g * 3 + GROUPS[g] - 1  # last m-tile of group
                            if g_hi not in (mA, mB):
                                continue  # group not fully written yet
                            pt_g[g] = work.tile(
                                [128, GROUPS[g] * CH], F16, tag="pt",
                                name=f"pt_{b}_{i}{j}_{c}_{g}",
                            )
                            nc.scalar.activation(
                                pt_g[g], st_g[g],
                                mybir.ActivationFunctionType.Exp, scale=SCALE,
                            )
                            for p in range(GROUPS[g]):
                                t = g * 3 + p
                                nc.tensor.matmul(
                                    out_ps,
                                    lhsT=(xo_sb[j][:, t, :]),
                                    rhs=(pt_g[g][:, p * CH : (p + 1) * CH]),
                                    start=(t == 0),
                                    stop=(t == NT - 1),
                                    skip_group_check=True,
                                )
                            if g == 0:
                                flush_pending()
                    # out_ps rows 0-63 = unnormalized out^T, row 64 = 3*Z.
                    # Copy to SBUF now (frees the PSUM bank for the next
                    # chunk); defer the transpose/normalize to flush_pending.
                    osb = work.tile([D + 1, CH], F32, tag="osb", name=f"osb_{b}_{i}{j}_{c}")
                    nc.vector.tensor_copy(out=osb, in_=out_ps)

                    def normalize(osb=osb, b=b, i=i, j=j, c=c, acc=acc):
                        otp = psum.tile(
                            [128, 4, D + 1], F32, tag="otp", bufs=1,
                            name=f"otp_{b}_{i}{j}_{c}",
                        )
                        for t in range(4):
                            nc.tensor.transpose(
                                otp[:, t, :],
                                osb[:, t * 128 : (t + 1) * 128],
                                id_sb[0 : D + 1, 0 : D + 1],
                            )
                        rz = work.tile([128, 4], F32, tag="rz", name=f"rz_{b}_{i}{j}_{c}")
                        nc.vector.reciprocal(rz, otp[:, :, D])
                        res = work.tile([128, 4, D], F32, tag="res", name=f"res_{b}_{i}{j}_{c}")
                        for t in range(4):
                            nc.vector.tensor_scalar_mul(
                                res[:, t, :], otp[:, t, 0:D], rz[:, t : t + 1]
                            )
                        nc.vector.tensor_tensor(
                            out=acc[:, c * 4 : (c + 1) * 4, :],
                            in0=acc[:, c * 4 : (c + 1) * 4, :],
                            in1=res,
                            op=mybir.AluOpType.add,
                        )

                    pending.append(normalize)
            flush_pending()
            nc.sync.dma_start(
                out=out_d[b].rearrange("(t p) d -> p t d", p=128), in_=acc
            )
    nc.finalize()  # Bacc: split multi-waits, alloc regs, etc.
    return nc


def _prep(xs, Ws):
    """Host-side input prep: Q/K projections + layout shuffles."""
    qts, kts, xos = [], [], []
    for m in range(3):
        x = np.ascontiguousarray(xs[m], dtype=np.float32)  # [B, N, D]
        W = np.asarray(Ws[m], dtype=np.float32)
        Q = x @ W  # [B, N, D]
        K = x @ W.T
        QT = np.ascontiguousarray(Q.transpose(0, 2, 1))  # [B, D, N]
        KT = np.ascontiguousarray(K.transpose(0, 2, 1))
        qts.append(np.concatenate([QT, QT], axis=1).astype(np.float16))  # [B, 128, N]
        kts.append(np.concatenate([KT, KT], axis=1).astype(np.float16))
        xo = np.full((B, 128, NT, D + 1), 3.0, dtype=np.float16)
        # xo[b, p, t, :64] = x[b, t*128 + p, :]; col 64 stays 3.0 (folds the
        # mean over 3 modalities into the softmax normalization).
        xo[..., :D] = x.reshape(B, NT, 128, D).transpose(0, 2, 1, 3).astype(np.float16)
        xos.append(xo)
    return qts, kts, xos


def kernel(x_rna, x_cnv, x_clinical, W_rna, W_cnv, W_clinical):
    global last_results
    xs = [x_rna, x_cnv, x_clinical]
    Ws = [W_rna, W_cnv, W_clinical]
    qts, kts, xos = _prep(xs, Ws)
    ident = np.eye(128, dtype=np.float32)

    if "nc" not in _cache:
        _cache["nc"] = _build()
    nc = _cache["nc"]

    in_maps = []
    for c in range(NCORES):
        sl = slice(c * BPC, (c + 1) * BPC)
        m = {"ident": ident}
        for mod in range(3):
            m[f"qt{mod}"] = np.ascontiguousarray(qts[mod][sl])
            m[f"kt{mod}"] = np.ascontiguousarray(kts[mod][sl])
            m[f"xo{mod}"] = np.ascontiguousarray(xos[mod][sl])
        in_maps.append(m)

    last_results = run_bass_kernel_spmd(
        nc,
        in_maps,
        list(range(NCORES)),
        trace=bool(os.environ.get("BASS_TRACE")),
    )
    out = np.concatenate([r["out"] for r in last_results.results], axis=0)
    return out


# revision 11
# speedup vs baseline: 1.0270x; 1.0228x over previous
"""CrossModalAttention TRN2 kernel.

Computation (per batch b):
  Q_m = x_m @ W_m ; K_m = x_m @ W_m^T   (m in {rna, cnv, clinical})
  out  = mean_i( sum_{j!=i} softmax(Q_i K_j^T / 8) @ x_j )

Strategy:
  - Pure data parallel: batch dim 16 sharded 2-per-core across 8 NeuronCores.
  - Tiny Q/K projections (1.6 GFLOP total) are precomputed on host; the device
    kernel runs the O(N^2) attention (103 GFLOP + 403M exps), which dominates.
  - Scores are computed transposed (ST[m, n] tiles) so the PV matmul contracts
    over m on the partition dim.  K=64 contraction -> two concurrent
    tile_position matmuls (rows 0-63 / 64-127) with Q,K duplicated across
    both partition halves.
  - exp on ScalarE reads score tiles straight from PSUM ([128, 1024] per
    instruction), with the 1/sqrt(64) scale folded into the activation.
    No max-subtraction: scores are ~N(0,1), max over 4e8 samples < 7, far
    from fp32 exp overflow.
  - Softmax denominator rides along as a 65th "feature" column of x_j set to
    3.0, so the PV matmul accumulates [65, 512] = [out^T ; 3*Z].  The 3.0
    folds the final mean-over-3-modalities into the existing normalization.
  - out^T chunks are PE-transposed back to [n, d] layout, normalized with a
    per-partition reciprocal scalar on VectorE, and accumulated over the 6
    (i, j) pairs in SBUF.
"""

import os

import numpy as np

import concourse.bass as bass
import concourse.bacc as bacc
import concourse.tile as tile
from concourse import mybir
from concourse.bass_utils import run_bass_kernel_spmd

B, N, D = 16, 2048, 64
NCORES = 8
BPC = B // NCORES  # batches per core
NT = N // 128  # 16 row-tiles of 128
CH = 512  # n-chunk (PSUM bank)
NCH = N // CH  # 4
PAIRS = [(i, j) for i in range(3) for j in range(3) if i != j]
SCALE = 1.0 / 8.0  # 1/sqrt(D)
F32 = mybir.dt.float32
F16 = mybir.dt.float16  # matmul operand dtype: 1 cyc/row, 10-bit mantissa

_cache = {}
last_results = None  # BassKernelResults of the most recent run (for test.py)


def _build():
    nc = bacc.Bacc()
    qt_d = [
        nc.declare_dram_parameter(f"qt{m}", [BPC, 128, N], F16, isOutput=False)
        for m in range(3)
    ]
    kt_d = [
        nc.declare_dram_parameter(f"kt{m}", [BPC, 128, N], F16, isOutput=False)
        for m in range(3)
    ]
    xo_d = [
        nc.declare_dram_parameter(f"xo{m}", [BPC, 128, NT, D + 1], F16, isOutput=False)
        for m in range(3)
    ]
    id_d = nc.declare_dram_parameter("ident", [128, 128], F32, isOutput=False)
    out_d = nc.declare_dram_parameter("out", [BPC, N, D], F32, isOutput=True)

    from contextlib import ExitStack

    with tile.TileContext(nc) as tc, ExitStack() as ctx:
        singles = ctx.enter_context(tc.tile_pool(name="singles", bufs=1))
        big = ctx.enter_context(tc.tile_pool(name="big", bufs=2))
        work = ctx.enter_context(tc.tile_pool(name="work", bufs=3))
        psum = ctx.enter_context(tc.tile_pool(name="psum", bufs=2, space="PSUM"))

        id_sb = singles.tile([128, 128], F32)
        nc.sync.dma_start(out=id_sb, in_=id_d[:, :])
        # Warm up the ACT engine: absorb the exp-table load and the const
        # bias-AP DMA wait into one early instruction, so the first real exp
        # (which also waits on PE) stays within the single ACT wait slot.
        warm = singles.tile([128, 1], F32)
        bias0 = nc.const_aps.scalar_like(0.0, warm[:, 0:1])
        nc.scalar.activation(warm, bias0, mybir.ActivationFunctionType.Exp)

        for b in range(BPC):
            qt_sb, kt_sb, xo_sb = [None] * 3, [None] * 3, [None] * 3
            for m in range(3):
                qt_sb[m] = big.tile([128, N], F16, tag=f"qt{m}", name=f"qt{m}_{b}")
                kt_sb[m] = big.tile([128, N], F16, tag=f"kt{m}", name=f"kt{m}_{b}")
                xo_sb[m] = big.tile(
                    [128, NT, D + 1], F16, tag=f"xo{m}", name=f"xo{m}_{b}"
                )
            # Issue the first pair's ((0,1)) inputs first so compute can start
            # before the remaining loads land.
            for m, t_sb, t_d in (
                (0, qt_sb, qt_d), (1, kt_sb, kt_d), (1, xo_sb, xo_d),
                (1, qt_sb, qt_d), (2, kt_sb, kt_d), (2, xo_sb, xo_d),
                (2, qt_sb, qt_d), (0, kt_sb, kt_d), (0, xo_sb, xo_d),
            ):
                nc.sync.dma_start(out=t_sb[m], in_=t_d[m][b])
            acc = big.tile([128, NT, D], F32, tag="acc", name=f"acc_{b}")
            nc.vector.memset(acc, 0.0)
            # Flat schedule of score-tile "groups" (up to 3 m-tiles -> one
            # exp instruction).  Score matmuls are emitted ONE GROUP AHEAD
            # of the exp/PV stream so the PE queue always has the next
            # group's scores ready before the chunk-tail PV/transpose work —
            # otherwise the exp stream stalls ~1.1us at every chunk boundary.
            GROUPS = [3, 3, 3, 3, 3, 1]
            NG = len(GROUPS)
            sched = [
                (i, j, c, g) for (i, j) in PAIRS for c in range(NCH)
                for g in range(NG)
            ]
            pending = []  # deferred per-chunk normalize work

            def flush_pending():
                while pending:
                    pending.pop(0)()

            st_tiles = {}

            def emit_st(idx):
                i, j, c, g = sched[idx]
                stt = psum.tile(
                    [128, GROUPS[g] * CH], F32, tag="st",
                    name=f"st_{b}_{i}{j}_{c}_{g}",
                )
                st_tiles[idx] = stt
                for p in range(GROUPS[g]):
                    t = 3 * g + p
                    h = (t % 2) * 64  # alternate PE row halves -> concurrent
                    nc.tensor.matmul(
                        stt[:, p * CH : (p + 1) * CH],
                        lhsT=kt_sb[j][h : h + 64, t * 128 : (t + 1) * 128],
                        rhs=qt_sb[i][h : h + 64, c * CH : (c + 1) * CH],
                        start=True,
                        stop=True,
                    )

            emit_st(0)
            out_ps = None
            for idx, (i, j, c, g) in enumerate(sched):
                if g == 0:
                    out_ps = psum.tile(
                        [D + 1, CH], F32, tag="out", bufs=1,
                        name=f"o_{b}_{i}{j}_{c}",
                    )
                if idx + 1 < len(sched):
                    emit_st(idx + 1)
                stt = st_tiles.pop(idx)
                ptt = work.tile(
                    [128, GROUPS[g] * CH], F16, tag="pt",
                    name=f"pt_{b}_{i}{j}_{c}_{g}",
                )
                nc.scalar.activation(
                    ptt, stt, mybir.ActivationFunctionType.Exp, scale=SCALE
                )
                for p in range(GROUPS[g]):
                    t = 3 * g + p
                    nc.tensor.matmul(
                        out_ps,
                        lhsT=(xo_sb[j][:, t, :]),
                        rhs=(ptt[:, p * CH : (p + 1) * CH]),
                        start=(t == 0),
                        stop=(t == NT - 1),
                        skip_group_check=True,
                    )
                if g == 0:
                    flush_pending()
                if g == NG - 1:
                    # out_ps rows 0-63 = unnormalized out^T, row 64 = 3*Z.
                    # Copy to SBUF now (frees the PSUM bank for the next
                    # chunk); defer transpose/normalize to flush_pending.
                    osb = work.tile(
                        [D + 1, CH], F32, tag="osb", name=f"osb_{b}_{i}{j}_{c}"
                    )
                    nc.vector.tensor_copy(out=osb, in_=out_ps)

                    def normalize(osb=osb, b=b, i=i, j=j, c=c, acc=acc):
                        otp = psum.tile(
                            [128, 4, D + 1], F32, tag="otp", bufs=1,
                            name=f"otp_{b}_{i}{j}_{c}",
                        )
                        for t in range(4):
                            nc.tensor.transpose(
                                otp[:, t, :],
                                osb[:, t * 128 : (t + 1) * 128],
                                id_sb[0 : D + 1, 0 : D + 1],
                            )
                        rz = work.tile([128, 4], F32, tag="rz", name=f"rz_{b}_{i}{j}_{c}")
                        nc.vector.reciprocal(rz, otp[:, :, D])
                        res = work.tile([128, 4, D], F32, tag="res", name=f"res_{b}_{i}{j}_{c}")
                        for t in range(4):
                            nc.vector.tensor_scalar_mul(
                                res[:, t, :], otp[:, t, 0:D], rz[:, t : t + 1]
                            )
                        nc.vector.tensor_tensor(
                            out=acc[:, c * 4 : (c + 1) * 4, :],
                            in0=acc[:, c * 4 : (c + 1) * 4, :],
                            in1=res,
                            op=mybir.AluOpType.add,
                        )

                    pending.append(normalize)
            flush_pending()
            nc.sync.dma_start(
                out=out_d[b].rearrange("(t p) d -> p t d", p=128), in_=acc
            )
    nc.finalize()  # Bacc: split multi-waits, alloc regs, etc.
    return nc


def _prep(xs, Ws):
    """Host-side input prep: Q/K projections + layout shuffles."""
    qts, kts, xos = [], [], []
    for m in range(3):
        x = np.ascontiguousarray(xs[m], dtype=np.float32)  # [B, N, D]
        W = np.asarray(Ws[m], dtype=np.float32)
        Q = x @ W  # [B, N, D]
        K = x @ W.T
        QT = np.ascontiguousarray(Q.transpose(0, 2, 1))  # [B, D, N]
        KT = np.ascontiguousarray(K.transpose(0, 2, 1))
        qts.append(np.concatenate([QT, QT], axis=1).astype(np.float16))  # [B, 128, N]
        kts.append(np.concatenate([KT, KT], axis=1).astype(np.float16))
        xo = np.full((B, 128, NT, D + 1), 3.0, dtype=np.float16)
        # xo[b, p, t, :64] = x[b, t*128 + p, :]; col 64 stays 3.0 (folds the
        # mean over 3 modalities into the softmax normalization).
        xo[..., :D] = x.reshape(B, NT, 128, D).transpose(0, 2, 1, 3).astype(np.float16)
        xos.append(xo)
    return qts, kts, xos


def kernel(x_rna, x_cnv, x_clinical, W_rna, W_cnv, W_clinical):
    global last_results
    xs = [x_rna, x_cnv, x_clinical]
    Ws = [W_rna, W_cnv, W_clinical]
    qts, kts, xos = _prep(xs, Ws)
    ident = np.eye(128, dtype=np.float32)

    if "nc" not in _cache:
        _cache["nc"] = _build()
    nc = _cache["nc"]

    in_maps = []
    for c in range(NCORES):
        sl = slice(c * BPC, (c + 1) * BPC)
        m = {"ident": ident}
        for mod in range(3):
            m[f"qt{mod}"] = np.ascontiguousarray(qts[mod][sl])
            m[f"kt{mod}"] = np.ascontiguousarray(kts[mod][sl])
            m[f"xo{mod}"] = np.ascontiguousarray(xos[mod][sl])
        in_maps.append(m)

    last_results = run_bass_kernel_spmd(
        nc,
        in_maps,
        list(range(NCORES)),
        trace=bool(os.environ.get("BASS_TRACE")),
    )
    out = np.concatenate([r["out"] for r in last_results.results], axis=0)
    return out
